# revision 3
# baseline (speedup 1.0000x reference)
"""Trainium2 Bass kernel for CellPathwayPoolingAggregator (segment mean).

out[b, p] = (1/segment_sizes[p]) * sum_{k: segment_ids[k]==p} x[b, flat_indices[k]]

Strategy (8 cores, sharded by contiguous pathway ranges):
  - Host: split the 1000 pathways into 8 contiguous ranges (<=128 pathways
    each) balancing per-core unique-gene counts; dedupe each core's gene
    rows (S carries counts). Pack each core's working set as a dense
    per-core DRAM tensor xtp (128, T, 2048) fp16 with [p, t, :] = the
    (t*128+p)-th deduped gene row. All indexing happens on host, so the
    device does plain dense HWDGE loads (16KB descriptors, no SWDGE
    descriptor-generation bottleneck, no index uploads).
  - Device (per core): chunked nc.sync.dma_start loads pull 4 K-tiles
    (128, 4, 2048) = 2MB per instruction from DRAM into SBUF. A PE matmul
    with the per-core count matrix S (128 k-rows x 128 local pathways,
    stationary) streams the loaded rows and accumulates pathway x batch
    sums into 4 PSUM banks.
  - DVE/ACT scale rows by 1/segment_sizes, DMA stores the (128, 2048) f32
    transposed output slice; host reassembles/transposes.

The single SPMD program is uniform across cores (T_max tiles each,
zero-padded); per-core data lives in xtp/smat/invsz bindings.
"""

import sys

import numpy as np

_TRN_REPO = "/opt/trn_rl_repo"
if _TRN_REPO not in sys.path:
    sys.path.insert(0, _TRN_REPO)

import concourse.bass as bass  # noqa: F401
import concourse.mybir as mybir
import concourse.tile as tile
from concourse import bacc
from concourse.bass_utils import run_bass_kernel_spmd

B, G, P = 2048, 10000, 1000
NCORES = 8
PC = 128          # max pathways per core (psum partition dim)
NB = B // 512     # matmul N-slices per K-tile (4 banks of 512 f32)
CH_MAX = 4        # K-tiles per dense load (16KB per-partition descriptors)


def _chunk_list(T):
    """Split T tiles into load chunks: small first chunk (fast first MM),
    middle chunks of CH_MAX, small last chunk (short PE trail)."""
    if T <= 2:
        return [T]
    chunks = [2]
    rem = T - 2
    while rem > CH_MAX:
        chunks.append(CH_MAX)
        rem -= CH_MAX
    if rem:
        chunks.append(rem)
    return chunks


def _split_ranges(seg_sorted, idx_sorted):
    """Contiguous pathway ranges, <=128 pathways each, minimizing the max
    per-core count of UNIQUE genes (which sets T and hence DMA/PE work)."""
    seg_starts = np.searchsorted(seg_sorted, np.arange(P + 1), side="left")

    def feasible(U):
        bounds = [0]
        for c in range(NCORES):
            lo_p = bounds[-1]
            if lo_p >= P:
                return None
            best = lo_p + 1
            hi_cap = min(P, lo_p + PC)
            lo_e = seg_starts[lo_p]
            for hi_p in range(lo_p + 1, hi_cap + 1):
                nu = len(np.unique(idx_sorted[lo_e : seg_starts[hi_p]]))
                if nu <= U:
                    best = hi_p
                else:
                    break
            bounds.append(best)
        return bounds if bounds[-1] >= P else None

    lo_t, hi_t = 1, (len(idx_sorted) + 127) // 128 + 1
    best_bounds = None
    while lo_t <= hi_t:
        mid = (lo_t + hi_t) // 2
        b = feasible(mid * 128)
        if b is not None:
            best_bounds = b
            hi_t = mid - 1
        else:
            lo_t = mid + 1
    if best_bounds is None:
        best_bounds = list(
            np.minimum(np.arange(NCORES + 1) * ((P + NCORES - 1) // NCORES), P)
        )
    best_bounds[-1] = P
    return best_bounds


def _build_schedule(flat_indices, segment_ids):
    seg = np.asarray(segment_ids, dtype=np.int64)
    idx = np.asarray(flat_indices, dtype=np.int64)
    order = np.argsort(seg, kind="stable")
    seg = seg[order]
    idx = idx[order]

    bounds = _split_ranges(seg, idx)
    cores = []
    for c in range(NCORES):
        lo_p, hi_p = bounds[c], bounds[c + 1]
        lo = np.searchsorted(seg, lo_p, side="left")
        hi = np.searchsorted(seg, hi_p, side="left")
        # Deduplicate gene rows within the core: each distinct gene appears
        # once in the packed tensor; S accumulates per-(gene,pathway) counts.
        uidx, inv = np.unique(idx[lo:hi], return_inverse=True)
        cores.append((lo_p, hi_p, uidx, inv, seg[lo:hi] - lo_p))

    T = max(1, max((len(u) + 127) // 128 for _, _, u, _, _ in cores))
    Kpad = T * 128

    uidx_pads, s_sbs = [], []
    for lo_p, hi_p, uidx, inv, cols in cores:
        nu = len(uidx)
        uidx_pads.append(np.concatenate([uidx, np.zeros(Kpad - nu, np.int64)]))
        S = np.zeros((Kpad, PC), np.float32)
        np.add.at(S, (inv, cols), 1.0)
        S = S.astype(np.float16)
        s_sbs.append(
            np.ascontiguousarray(
                S.reshape(T, 128, PC).transpose(1, 0, 2).reshape(128, T * PC)
            )
        )
    return bounds, uidx_pads, s_sbs, T


def _build_program(T):
    nc = bacc.Bacc(
        "TRN2",
        target_bir_lowering=False,
        debug=False,
        num_devices=NCORES,
    )
    f16, f32 = mybir.dt.float16, mybir.dt.float32

    # Packed working set: [p, t, :] = gene row (t*128 + p) of this core's
    # deduped schedule. Per-partition runs are contiguous across t, so a
    # chunk load [:, t0:t0+ch, :] is one descriptor of ch*4KB per partition.
    xtp_d = nc.dram_tensor("xtp", [128, T, B], f16, kind="ExternalInput")
    s_d = nc.dram_tensor("smat", [128, T * PC], f16, kind="ExternalInput")
    inv_d = nc.dram_tensor("invsz", [128, 1], f32, kind="ExternalInput")
    out_d = nc.dram_tensor("out", [PC, B], f32, kind="ExternalOutput")

    chunks = _chunk_list(T)

    with tile.TileContext(nc) as tc:
        with (
            tc.tile_pool(name="const", bufs=1) as cpool,
            tc.tile_pool(name="load", bufs=3) as gpool,
            tc.tile_pool(name="psum", bufs=1, space="PSUM") as ppool,
            tc.tile_pool(name="outp", bufs=1) as opool,
        ):
            # smat/invsz on the Activation HWDGE queue; xtp chunks on Sync.
            s_sb = cpool.tile([128, T * PC], f16, tag="smat")
            nc.scalar.dma_start(s_sb[:], s_d.ap())
            inv_sb = cpool.tile([128, 1], f32, tag="invsz")
            nc.scalar.dma_start(inv_sb[:], inv_d.ap())

            psb = [
                ppool.tile([128, 512], f32, tag=f"ps{n}", name=f"ps{n}")
                for n in range(NB)
            ]

            t0 = 0
            for ci, ch in enumerate(chunks):
                gt = gpool.tile([128, ch, B], f16, tag="gt", name=f"gt{ci}")
                nc.sync.dma_start(gt[:], xtp_d.ap()[:, t0 : t0 + ch, :])
                last_chunk = ci == len(chunks) - 1
                # Middle chunks: tiles outer (PE consumes in load order).
                # Last chunk: banks outer so early banks finish (and start
                # evicting) while later banks' matmuls still stream.
                if not last_chunk:
                    for tl in range(ch):
                        tt = t0 + tl
                        for n in range(NB):
                            nc.tensor.matmul(
                                psb[n][:],
                                s_sb[:, tt * PC : (tt + 1) * PC],
                                gt[:, tl, n * 512 : (n + 1) * 512],
                                start=(tt == 0),
                                stop=False,
                            )
                else:
                    for n in range(NB):
                        for tl in range(ch):
                            tt = t0 + tl
                            nc.tensor.matmul(
                                psb[n][:],
                                s_sb[:, tt * PC : (tt + 1) * PC],
                                gt[:, tl, n * 512 : (n + 1) * 512],
                                start=(tt == 0),
                                stop=(tt == T - 1),
                            )
                t0 += ch

            # Per-bank eviction + store, alternating DVE and ACT; stores
            # alternate Sync/Scalar HWDGE queues to halve desc-gen serial.
            for n in range(NB):
                ot = opool.tile([128, 512], f32, tag=f"ot{n}", name=f"ot{n}")
                if n % 2 == 0:
                    nc.vector.tensor_scalar_mul(ot[:], psb[n][:], inv_sb[:])
                    nc.sync.dma_start(
                        out_d.ap()[:, n * 512 : (n + 1) * 512], ot[:]
                    )
                else:
                    nc.scalar.activation(
                        ot[:],
                        psb[n][:],
                        mybir.ActivationFunctionType.Identity,
                        scale=inv_sb[:],
                    )
                    nc.scalar.dma_start(
                        out_d.ap()[:, n * 512 : (n + 1) * 512], ot[:]
                    )
    return nc


def _prepare(gene_set_features, flat_indices, segment_ids, segment_sizes):
    bounds, uidx_pads, s_sbs, T = _build_schedule(flat_indices, segment_ids)
    nc = _build_program(T)
    nc.compile()

    x = np.asarray(gene_set_features, dtype=np.float32)
    xt16 = np.ascontiguousarray(x.T.astype(np.float16))  # (G, B)
    sizes = np.asarray(segment_sizes, dtype=np.float32)

    in_maps = []
    for c in range(NCORES):
        lo_p, hi_p = bounds[c], bounds[c + 1]
        inv = np.ones((128, 1), np.float32)
        inv[: hi_p - lo_p, 0] = 1.0 / sizes[lo_p:hi_p]
        xtp = np.ascontiguousarray(
            xt16[uidx_pads[c]].reshape(T, 128, B).transpose(1, 0, 2)
        )
        in_maps.append({"xtp": xtp, "smat": s_sbs[c], "invsz": inv})
    return nc, in_maps, bounds


def kernel(gene_set_features, flat_indices, segment_ids, segment_sizes, _res_hook=None):
    nc, in_maps, bounds = _prepare(
        gene_set_features, flat_indices, segment_ids, segment_sizes
    )
    res = run_bass_kernel_spmd(nc, in_maps, list(range(NCORES)))
    if _res_hook is not None:
        _res_hook(res)
    outT = np.empty((P, B), np.float32)
    for c in range(NCORES):
        lo_p, hi_p = bounds[c], bounds[c + 1]
        outT[lo_p:hi_p] = np.asarray(res.results[c]["out"])[: hi_p - lo_p]
    return np.ascontiguousarray(outT.T)


# revision 5
# speedup vs baseline: 1.2164x; 1.2164x over previous
"""Trainium2 Bass kernel for CellPathwayPoolingAggregator (segment mean).

out[b, p] = (1/segment_sizes[p]) * sum_{k: segment_ids[k]==p} x[b, flat_indices[k]]

Strategy (8 cores = 2 pathway ranges x 4 batch shards):
  - Host: split the 1000 pathways into 2 contiguous ranges (<=512 pathways,
    4 pathway-tiles of <=128 each) balancing unique-gene counts. Per range,
    dedupe genes and sort them by pathway-tile signature so each K-tile of
    128 genes touches few pathway-tiles. Pack each core's working set as a
    dense DRAM tensor xtp (128, T, 512) fp16 = its range's deduped gene rows
    restricted to its batch quarter. All indexing happens on host; the
    device does plain dense HWDGE loads (8KB per-partition descriptors).
  - The two ranges share one uniform MM pattern (per-tile union of both
    ranges' pathway-tile lists) so the single SPMD program fits all cores;
    a core's S blocks are zero where its range doesn't touch the tile.
  - Device (per core): chunked dense loads (8 K-tiles = 1MB per DMA) feed
    PE matmuls with per-block count matrices S (128 genes x 128 pathways,
    stationary, fp16) accumulating into 4 PSUM banks (one per pathway-tile,
    128 pathways x 512 batch). S slices are interleaved with data chunks on
    the same HWDGE ring so the first matmul starts early. A few warm-up
    matmuls on a zeroed tile flip the PE HAM clock gate to full rate before
    real work arrives.
  - DVE/ACT scale pathway rows by 1/segment_sizes as each bank's last
    matmul retires (signature sort staggers bank completions), DMA stores
    (128, 512) f32 slices; host reassembles/transposes.
"""

import sys
from collections import Counter

import numpy as np

_TRN_REPO = "/opt/trn_rl_repo"
if _TRN_REPO not in sys.path:
    sys.path.insert(0, _TRN_REPO)

import concourse.bass as bass  # noqa: F401
import concourse.mybir as mybir
import concourse.tile as tile
from concourse import bacc
from concourse.bass_utils import run_bass_kernel_spmd

B, G, P = 2048, 10000, 1000
NCORES = 8
NPT = 4           # pathway tiles per range
BQ = 512          # batch columns per core (B / 4 shards)
CH_MAX = 8        # K-tiles per dense load chunk (8KB/partition descriptors)
N_WARM = 8        # PE warm-up matmuls


def _chunk_list(T):
    if T <= 2:
        return [T]
    chunks = [2]
    rem = T - 2
    while rem > CH_MAX:
        chunks.append(CH_MAX)
        rem -= CH_MAX
    if rem:
        chunks.append(rem)
    return chunks


def _build_schedule(flat_indices, segment_ids):
    seg = np.asarray(segment_ids, dtype=np.int64)
    idx = np.asarray(flat_indices, dtype=np.int64)
    order = np.argsort(seg, kind="stable")
    seg, idx = seg[order], idx[order]
    seg_starts = np.searchsorted(seg, np.arange(P + 1))

    # range boundary balancing unique-gene counts (range sizes <= NPT*128)
    best, best_cost = None, None
    for b in range(P - NPT * 128, NPT * 128 + 1):
        uA = len(np.unique(idx[: seg_starts[b]]))
        uB = len(np.unique(idx[seg_starts[b] :]))
        cost = max(uA, uB)
        if best_cost is None or cost < best_cost:
            best, best_cost = b, cost
    bounds = [0, best, P]

    ranges = []
    for R in range(2):
        lo_p, hi_p = bounds[R], bounds[R + 1]
        lo, hi = seg_starts[lo_p], seg_starts[hi_p]
        genes = idx[lo:hi]
        lseg = seg[lo:hi] - lo_p
        pt = lseg // 128
        sig = {}
        for g, p_ in zip(genes.tolist(), pt.tolist()):
            sig.setdefault(g, set()).add(p_)
        genes_sorted = sorted(sig.keys(), key=lambda g: (tuple(sorted(sig[g])), g))
        ranges.append((lo_p, hi_p, genes, lseg, sig, genes_sorted))

    T2 = max((len(r[5]) + 127) // 128 for r in ranges)
    Kpad = T2 * 128

    tile_pts, gene_pads = [], []
    for lo_p, hi_p, genes, lseg, sig, gs in ranges:
        gpad = gs + [-1] * (Kpad - len(gs))
        gene_pads.append(gpad)
        L = []
        for t in range(T2):
            un = set()
            for g in gpad[t * 128 : (t + 1) * 128]:
                if g >= 0:
                    un.update(sig[g])
            L.append(sorted(un))
        tile_pts.append(L)

    pattern = [sorted(set(tile_pts[0][t]) | set(tile_pts[1][t])) for t in range(T2)]
    blocks = [(t, p_) for t in range(T2) for p_ in pattern[t]]
    M = len(blocks)
    first_touch, last_touch = {}, {}
    for m, (t, p_) in enumerate(blocks):
        first_touch.setdefault(p_, m)
        last_touch[p_] = m

    block_of = {tp: m for m, tp in enumerate(blocks)}
    smats = []
    for R, (lo_p, hi_p, genes, lseg, sig, gs) in enumerate(ranges):
        gpad = gene_pads[R]
        pos = {g: j for j, g in enumerate(gpad) if g >= 0}
        S = np.zeros((128, M * 128), np.float32)
        cnt = Counter(zip(genes.tolist(), lseg.tolist()))
        for (g, lp), c in cnt.items():
            j = pos[g]
            m = block_of[(j // 128, lp // 128)]
            S[j % 128, m * 128 + (lp % 128)] += c
        smats.append(S.astype(np.float16))

    chunks = _chunk_list(T2)
    mranges, t0 = [], 0
    for ch in chunks:
        m0 = sum(len(pattern[t]) for t in range(t0))
        m1 = m0 + sum(len(pattern[t]) for t in range(t0, t0 + ch))
        mranges.append((m0, m1))
        t0 += ch

    return dict(
        bounds=bounds, T2=T2, blocks=blocks,
        first_touch=first_touch, last_touch=last_touch,
        gene_pads=gene_pads, smats=smats, chunks=chunks, mranges=mranges,
    )


def _build_program(sch):
    nc = bacc.Bacc(
        "TRN2",
        target_bir_lowering=False,
        debug=False,
        num_devices=NCORES,
    )
    f16, f32 = mybir.dt.float16, mybir.dt.float32

    T2 = sch["T2"]
    blocks = sch["blocks"]
    chunks = sch["chunks"]
    mranges = sch["mranges"]
    first_touch, last_touch = sch["first_touch"], sch["last_touch"]

    xtp_d = nc.dram_tensor("xtp", [128, T2, BQ], f16, kind="ExternalInput")
    s_ds = [
        nc.dram_tensor(f"s{ci}", [128, (m1 - m0) * 128], f16, kind="ExternalInput")
        for ci, (m0, m1) in enumerate(mranges)
    ]
    inv_d = nc.dram_tensor("invsz", [128, NPT], f32, kind="ExternalInput")
    out_d = nc.dram_tensor("out", [NPT * 128, BQ], f32, kind="ExternalOutput")

    with tile.TileContext(nc) as tc:
        with (
            tc.tile_pool(name="const", bufs=1) as cpool,
            tc.tile_pool(name="warmp", bufs=1) as wpool,
            tc.tile_pool(name="psum", bufs=1, space="PSUM") as ppool,
            tc.tile_pool(name="outp", bufs=1) as opool,
        ):
            # PE warm-up: zeroed operands, separate PSUM bank. Runs while the
            # first S/x chunks stream in, flipping HAM to 8/8 early.
            warm_sb = wpool.tile([128, 640], f16, tag="warm")
            nc.gpsimd.memset(warm_sb[:], 0.0)
            warm_ps = ppool.tile([128, 512], f32, tag="wps", name="wps")
            for i in range(N_WARM):
                nc.tensor.matmul(
                    warm_ps[:],
                    warm_sb[:, 512:640],
                    warm_sb[:, 0:512],
                    start=(i == 0),
                    stop=(i == N_WARM - 1),
                )
            warm_out = wpool.tile([128, 512], f32, tag="warmo")
            nc.vector.tensor_copy(warm_out[:], warm_ps[:])

            inv_sb = cpool.tile([128, NPT], f32, tag="invsz")
            nc.scalar.dma_start(inv_sb[:], inv_d.ap())

            psb = [
                ppool.tile([128, 512], f32, tag=f"ps{n}", name=f"ps{n}")
                for n in range(NPT)
            ]
            s_sbs = []

            def evict(pt):
                ot = opool.tile([128, 512], f32, tag=f"ot{pt}", name=f"ot{pt}")
                if pt % 2 == 0:
                    nc.vector.tensor_scalar_mul(
                        ot[:], psb[pt][:], inv_sb[:, pt : pt + 1]
                    )
                    nc.sync.dma_start(
                        out_d.ap()[pt * 128 : (pt + 1) * 128, :], ot[:]
                    )
                else:
                    nc.scalar.activation(
                        ot[:],
                        psb[pt][:],
                        mybir.ActivationFunctionType.Identity,
                        scale=inv_sb[:, pt : pt + 1],
                    )
                    nc.scalar.dma_start(
                        out_d.ap()[pt * 128 : (pt + 1) * 128, :], ot[:]
                    )

            t0 = 0
            for ci, ch in enumerate(chunks):
                m0, m1 = mranges[ci]
                # S slice first, then the data chunk, on the same Sync HWDGE
                # ring: FIFO guarantees S arrives with (not after) the data.
                s_sb = cpool.tile(
                    [128, (m1 - m0) * 128], f16, tag=f"s{ci}", name=f"s{ci}"
                )
                nc.sync.dma_start(s_sb[:], s_ds[ci].ap())
                s_sbs.append(s_sb)
                gt = cpool.tile([128, ch, BQ], f16, tag=f"gt{ci}", name=f"gt{ci}")
                nc.sync.dma_start(gt[:], xtp_d.ap()[:, t0 : t0 + ch, :])

                for m in range(m0, m1):
                    tt, pt = blocks[m]
                    tl = tt - t0
                    nc.tensor.matmul(
                        psb[pt][:],
                        s_sb[:, (m - m0) * 128 : (m - m0 + 1) * 128],
                        gt[:, tl, :],
                        start=(m == first_touch[pt]),
                        stop=(m == last_touch[pt]),
                    )
                    if m == last_touch[pt]:
                        evict(pt)
                t0 += ch
    return nc


def _prepare(gene_set_features, flat_indices, segment_ids, segment_sizes):
    sch = _build_schedule(flat_indices, segment_ids)
    nc = _build_program(sch)
    nc.compile()

    x = np.asarray(gene_set_features, dtype=np.float32)
    xt16 = np.ascontiguousarray(x.T.astype(np.float16))  # (G, B)
    sizes = np.asarray(segment_sizes, dtype=np.float32)

    T2 = sch["T2"]
    bounds = sch["bounds"]
    # per-range shared arrays
    s_pieces, invs, gpads = [], [], []
    for R in range(2):
        lo_p, hi_p = bounds[R], bounds[R + 1]
        S = sch["smats"][R]
        s_pieces.append(
            [
                np.ascontiguousarray(S[:, m0 * 128 : m1 * 128])
                for (m0, m1) in sch["mranges"]
            ]
        )
        inv = np.ones((128, NPT), np.float32)
        for pt in range(NPT):
            lo_row = lo_p + pt * 128
            n = min(128, hi_p - lo_row)
            if n > 0:
                inv[:n, pt] = 1.0 / sizes[lo_row : lo_row + n]
        invs.append(inv)
        gpads.append(
            np.array([g if g >= 0 else 0 for g in sch["gene_pads"][R]], np.int64)
        )

    in_maps = []
    for c in range(NCORES):
        R, q = c // 4, c % 4
        xtp = np.ascontiguousarray(
            xt16[gpads[R], q * BQ : (q + 1) * BQ]
            .reshape(T2, 128, BQ)
            .transpose(1, 0, 2)
        )
        im = {"xtp": xtp, "invsz": invs[R]}
        for ci in range(len(sch["chunks"])):
            im[f"s{ci}"] = s_pieces[R][ci]
        in_maps.append(im)
    return nc, in_maps, sch


def _unshard(res, sch):
    bounds = sch["bounds"]
    out = np.empty((B, P), np.float32)
    for c in range(NCORES):
        R, q = c // 4, c % 4
        lo_p, hi_p = bounds[R], bounds[R + 1]
        o = np.asarray(res.results[c]["out"])  # (NPT*128, BQ)
        for pt in range(NPT):
            lo_row = lo_p + pt * 128
            n = min(128, hi_p - lo_row)
            if n > 0:
                out[q * BQ : (q + 1) * BQ, lo_row : lo_row + n] = o[
                    pt * 128 : pt * 128 + n
                ].T
    return np.ascontiguousarray(out)


def kernel(gene_set_features, flat_indices, segment_ids, segment_sizes, _res_hook=None):
    nc, in_maps, sch = _prepare(
        gene_set_features, flat_indices, segment_ids, segment_sizes
    )
    res = run_bass_kernel_spmd(nc, in_maps, list(range(NCORES)))
    if _res_hook is not None:
        _res_hook(res)
    return _unshard(res, sch)
